# revision 17
# baseline (speedup 1.0000x reference)
"""Trainium2 Bass kernel for nn_ActQuantWrapper (hadamard + per-token act quant + linear).

Math (per reference):
  z = (H_64 kron I_had) x / 8               -- FHT over 64 groups along feature dim
  sx[t] = clip(absmax(z[t,:])/127, 1e-5)    -- per-token scale
  xq = round(z/sx)*sx                        -- act quant-dequant
  out = xq @ weight.T + bias                 -- weight already per-channel quantized

Device strategy (8 cores, data-parallel over tokens, weight replicated):
  - qx = round(z/sx) and qw = round(w/sw) are integers in [-127,127]: exactly
    representable in bf16, so the matmul runs at full bf16 PE rate and the
    result is scaled by sx[t]*sw[o] afterward (near-exact numerics).
  - The weight arrives already quantized, so bf16(w * (1/sw)) lands exactly on
    the integer grid without explicit rounding.
  - Activation rounding uses the fp32 magic-number trick (+1.5*2^23, -1.5*2^23).
  - bf16 tensors are transposed k-major via DMA xbar transpose (single ring --
    concurrent transposes on both HWDGE rings corrupt data).
"""

import numpy as np

import concourse.bass as bass
import concourse.tile as tile
from concourse import bacc, mybir
from concourse.bass_utils import run_bass_kernel_spmd

F32 = mybir.dt.float32
BF16 = mybir.dt.bfloat16
MAGIC = 12582912.0  # 1.5 * 2**23: adding then subtracting rounds f32 to int (RNE)

N_CORES = 8
B, S, D_IN, D_OUT = 2, 2048, 4096, 4096
N_TOK = B * S
T_CORE = N_TOK // N_CORES  # 512 tokens per core
N_GROUPS = 64              # hadamard dimension (fixed by reference)


def build_kernel(n_tok, K, O, oc_size, trace_sim=False):
    """Build + compile the per-core kernel."""
    assert n_tok % 128 == 0 and K % 256 == 0 and O % oc_size == 0
    assert oc_size % 128 == 0
    n_tt = n_tok // 128     # token tiles
    n_kt = K // 128         # contraction tiles
    n_oc = O // oc_size     # output chunks
    ot_per_oc = oc_size // 128
    had_dim = K // N_GROUPS

    nc = bacc.Bacc("TRN2", target_bir_lowering=False, debug=False)
    x_d = nc.dram_tensor("x", [n_tok, K], F32, kind="ExternalInput")
    w_d = nc.dram_tensor("w", [O, K], F32, kind="ExternalInput")
    b_d = nc.dram_tensor("b", [O], F32, kind="ExternalInput")
    out_d = nc.dram_tensor("out", [n_tok, O], F32, kind="ExternalOutput")
    swsc_d = nc.dram_tensor("swsc", [O], F32)  # internal scratch for sw broadcast

    with tile.TileContext(nc, trace_sim=trace_sim) as tc:
        with (
            tc.tile_pool(name="xload", bufs=2) as xload,
            tc.tile_pool(name="xwork", bufs=1) as xwork,
            tc.tile_pool(name="qxp", bufs=1) as qxp,
            tc.tile_pool(name="wload", bufs=5) as wload,
            tc.tile_pool(name="wq", bufs=4) as wqp,
            tc.tile_pool(name="qwT", bufs=2) as qwTp,
            tc.tile_pool(name="bcast", bufs=1) as bcast,
            tc.tile_pool(name="outp", bufs=3) as outp,
            tc.tile_pool(name="consts", bufs=1) as consts,
            tc.tile_pool(name="psum", bufs=2, space=bass.MemorySpace.PSUM) as psum,
        ):
            qxT = consts.tile([128, n_kt, n_tok], BF16)
            sx_all = consts.tile([128, n_tt], F32)
            xsc = consts.tile([128, n_tt, 3], F32)   # m, r, r8 per token tile
            NQ = 4                                   # w streamed in quarter tiles
            KQ = K // NQ
            wsc = consts.tile([128, 16, NQ + 2], F32)  # q-maxes, mmax/rw, sw

            def w_prep(oc):
                """load + absmax + quant + transpose one output chunk -> qwT tile.

                Streamed at quarter-tile granularity so ~5 pieces pipeline
                through load->reduce->quant->transpose and handoff latency
                doesn't pace the chain."""
                qwT = qwTp.tile([128, n_kt, oc_size], BF16, tag="qwT")
                for j in range(ot_per_oc):
                    ot = oc * ot_per_oc + j
                    slot = ot % 16
                    wts = []
                    for q in range(NQ):
                        wt = wload.tile([128, KQ], F32, tag="wt")
                        nc.gpsimd.dma_start(
                            wt[:],
                            w_d.ap()[ot * 128:(ot + 1) * 128, q * KQ:(q + 1) * KQ],
                        )
                        nc.vector.tensor_reduce(
                            out=wsc[:, slot, q:q + 1], in_=wt[:],
                            axis=mybir.AxisListType.X,
                            op=mybir.AluOpType.max, apply_absolute_value=True,
                        )
                        wts.append(wt)
                    nc.vector.tensor_reduce(
                        out=wsc[:, slot, NQ:NQ + 1], in_=wsc[:, slot, 0:NQ],
                        axis=mybir.AxisListType.X, op=mybir.AluOpType.max,
                    )
                    sw = wsc[:, slot, NQ + 1:NQ + 2]
                    nc.vector.tensor_scalar(
                        out=sw, in0=wsc[:, slot, NQ:NQ + 1],
                        scalar1=float(np.float32(1.0) / np.float32(127.0)),
                        scalar2=1e-5,
                        op0=mybir.AluOpType.mult, op1=mybir.AluOpType.max,
                    )
                    nc.gpsimd.dma_start(swsc_d.ap()[ot * 128:(ot + 1) * 128], sw)
                    rw = wsc[:, slot, NQ:NQ + 1]  # overwrite mmax with 1/sw
                    nc.vector.reciprocal(rw, sw)
                    # weight is pre-quantized: w*rw lands within 5e-5 of an integer,
                    # so the bf16 output conversion rounds exactly onto the grid.
                    nktq = n_kt // NQ
                    for q in range(NQ):
                        qw = wqp.tile([128, KQ], BF16, tag="qw")
                        nc.scalar.activation(
                            out=qw[:], in_=wts[q][:],
                            func=mybir.ActivationFunctionType.Copy, bias=0.0, scale=rw,
                        )
                        nc.sync.dma_start_transpose(
                            qwT[:, q * nktq:(q + 1) * nktq, j * 128:(j + 1) * 128],
                            qw[:],
                        )
                # sw/bias broadcast tiles for this chunk
                swb = bcast.tile([128, oc_size], F32, tag="swb")
                src = swsc_d.ap()[oc * oc_size:(oc + 1) * oc_size]
                nc.gpsimd.dma_start(
                    out=swb[:],
                    in_=bass.AP(tensor=src.tensor, offset=src.offset,
                                ap=[[0, 128]] + list(src.ap)),
                )
                bb = bcast.tile([128, oc_size], F32, tag="bb")
                srcb = b_d.ap()[oc * oc_size:(oc + 1) * oc_size]
                nc.gpsimd.dma_start(
                    out=bb[:],
                    in_=bass.AP(tensor=srcb.tensor, offset=srcb.offset,
                                ap=[[0, 128]] + list(srcb.ap)),
                )
                return qwT, swb, bb

            def x_prep(tt):
                za = xload.tile([128, K], F32, tag="za")
                nc.sync.dma_start(za[:], x_d.ap()[tt * 128:(tt + 1) * 128, :])
                zb = xwork.tile([128, K], F32, tag="zb")
                bufs = [za, zb]
                for s in range(6):
                    src, dst = bufs[s % 2], bufs[(s + 1) % 2]
                    blk = had_dim << s
                    sv = src[:].rearrange("p (a c b) -> p a c b", c=2, b=blk)
                    dv = dst[:].rearrange("p (a c b) -> p a c b", c=2, b=blk)
                    nc.vector.tensor_add(dv[:, :, 0, :], sv[:, :, 0, :], sv[:, :, 1, :])
                    nc.vector.tensor_sub(dv[:, :, 1, :], sv[:, :, 0, :], sv[:, :, 1, :])
                m = xsc[:, tt, 0:1]
                nc.vector.tensor_reduce(
                    out=m, in_=za[:], axis=mybir.AxisListType.X,
                    op=mybir.AluOpType.max, apply_absolute_value=True,
                )
                nc.vector.tensor_scalar(
                    out=sx_all[:, tt:tt + 1], in0=m,
                    scalar1=float(np.float32(1.0) / np.float32(1016.0)),
                    scalar2=1e-5,
                    op0=mybir.AluOpType.mult, op1=mybir.AluOpType.max,
                )
                rx = xsc[:, tt, 1:2]
                nc.vector.reciprocal(rx, sx_all[:, tt:tt + 1])
                rx8 = xsc[:, tt, 2:3]
                nc.vector.tensor_scalar_mul(rx8, rx, 0.125)
                qtmp = xwork.tile([128, K], F32, tag="zb")
                nc.scalar.activation(
                    out=qtmp[:], in_=za[:], func=mybir.ActivationFunctionType.Copy,
                    bias=MAGIC, scale=rx8,
                )
                qx = qxp.tile([128, K], BF16, tag="qx")
                nc.vector.tensor_scalar_add(qx[:], qtmp[:], -MAGIC)
                nc.sync.dma_start_transpose(
                    qxT[:, :, tt * 128:(tt + 1) * 128], qx[:]
                )

            def mm_chunk(oc, qwT, swb, bb):
                for t in range(n_tt):
                    ps = psum.tile([128, oc_size], F32, tag=f"ps{t % 4}")
                    for k in range(n_kt):
                        nc.tensor.matmul(
                            ps[:],
                            qxT[:, k, t * 128:(t + 1) * 128],
                            qwT[:, k, :],
                            start=(k == 0), stop=(k == n_kt - 1),
                        )
                    o_sb = outp.tile([128, oc_size], F32, tag="osb")
                    nc.vector.scalar_tensor_tensor(
                        out=o_sb[:], in0=ps[:], scalar=sx_all[:, t:t + 1], in1=swb[:],
                        op0=mybir.AluOpType.mult, op1=mybir.AluOpType.mult,
                    )
                    nc.gpsimd.tensor_add(o_sb[:], o_sb[:], bb[:])
                    nc.gpsimd.dma_start(
                        out_d.ap()[t * 128:(t + 1) * 128,
                                   oc * oc_size:(oc + 1) * oc_size],
                        o_sb[:],
                    )

            # emission order: interleave x tiles with the first weight chunks so
            # the x transposes hit the xbar ring first and qwT0 follows closely.
            x_prep(0)
            pend = [w_prep(0)]
            if n_tt > 1:
                x_prep(1)
            if n_oc > 1:
                pend.append(w_prep(1))
            for tt in range(2, n_tt):
                x_prep(tt)
            for oc in range(n_oc):
                qwT, swb, bb = pend.pop(0)
                mm_chunk(oc, qwT, swb, bb)
                if oc + 2 < n_oc:
                    pend.append(w_prep(oc + 2))

    nc.compile()
    return nc


_CACHED = None


def _get_full_kernel():
    global _CACHED
    if _CACHED is None:
        _CACHED = build_kernel(T_CORE, D_IN, D_OUT, 512)
    return _CACHED


def kernel(x, weight, bias, had_dim):
    assert int(had_dim) == 64
    assert x.shape == (B, S, D_IN) and weight.shape == (D_OUT, D_IN)
    nc = _get_full_kernel()
    xf = np.ascontiguousarray(np.asarray(x).reshape(N_TOK, D_IN), dtype=np.float32)
    w = np.ascontiguousarray(np.asarray(weight), dtype=np.float32)
    bi = np.ascontiguousarray(np.asarray(bias), dtype=np.float32)
    in_maps = [
        {"x": xf[i * T_CORE:(i + 1) * T_CORE], "w": w, "b": bi}
        for i in range(N_CORES)
    ]
    res = run_bass_kernel_spmd(nc, in_maps, core_ids=list(range(N_CORES)))
    out = np.concatenate([r["out"] for r in res.results], axis=0)
    return out.reshape(B, S, D_OUT)


if __name__ == "__main__":
    rng = np.random.default_rng(0)
    x = rng.standard_normal((B, S, D_IN), dtype=np.float32)
    w = rng.standard_normal((D_OUT, D_IN), dtype=np.float32)
    b = rng.standard_normal(D_OUT).astype(np.float32)
    o = kernel(x, w, b, np.int64(64))
    print(o.shape, o.dtype)


# revision 18
# speedup vs baseline: 1.0644x; 1.0644x over previous
"""Trainium2 Bass kernel for nn_ActQuantWrapper (hadamard + per-token act quant + linear).

Math (per reference):
  z = (H_64 kron I_had) x / 8               -- FHT over 64 groups along feature dim
  sx[t] = clip(absmax(z[t,:])/127, 1e-5)    -- per-token scale
  xq = round(z/sx)*sx                        -- act quant-dequant
  out = xq @ weight.T + bias                 -- weight already per-channel quantized

Device strategy (8 cores, data-parallel over tokens, weight replicated):
  - qx = round(z/sx) and qw = round(w/sw) are integers in [-127,127]: exactly
    representable in bf16, so the matmul runs at full bf16 PE rate and the
    result is scaled by sx[t]*sw[o] afterward (near-exact numerics).
  - The weight arrives already quantized, so bf16(w * (1/sw)) lands exactly on
    the integer grid without explicit rounding.
  - Activation rounding uses the fp32 magic-number trick (+1.5*2^23, -1.5*2^23).
  - bf16 tensors are transposed k-major via DMA xbar transpose (single ring --
    concurrent transposes on both HWDGE rings corrupt data).
"""

import numpy as np

import concourse.bass as bass
import concourse.tile as tile
from concourse import bacc, mybir
from concourse.bass_utils import run_bass_kernel_spmd

F32 = mybir.dt.float32
BF16 = mybir.dt.bfloat16
MAGIC = 12582912.0  # 1.5 * 2**23: adding then subtracting rounds f32 to int (RNE)

N_CORES = 8
B, S, D_IN, D_OUT = 2, 2048, 4096, 4096
N_TOK = B * S
T_CORE = N_TOK // N_CORES  # 512 tokens per core
N_GROUPS = 64              # hadamard dimension (fixed by reference)


def build_kernel(n_tok, K, O, oc_size, trace_sim=False):
    """Build + compile the per-core kernel."""
    assert n_tok % 128 == 0 and K % 256 == 0 and O % oc_size == 0
    assert oc_size % 128 == 0
    n_tt = n_tok // 128     # token tiles
    n_kt = K // 128         # contraction tiles
    n_oc = O // oc_size     # output chunks
    ot_per_oc = oc_size // 128
    had_dim = K // N_GROUPS

    nc = bacc.Bacc("TRN2", target_bir_lowering=False, debug=False)
    x_d = nc.dram_tensor("x", [n_tok, K], F32, kind="ExternalInput")
    w_d = nc.dram_tensor("w", [O, K], F32, kind="ExternalInput")
    b_d = nc.dram_tensor("b", [O], F32, kind="ExternalInput")
    out_d = nc.dram_tensor("out", [n_tok, O], F32, kind="ExternalOutput")
    swsc_d = nc.dram_tensor("swsc", [O], F32)  # internal scratch for sw broadcast

    with tile.TileContext(nc, trace_sim=trace_sim) as tc:
        with (
            tc.tile_pool(name="xload", bufs=2) as xload,
            tc.tile_pool(name="xwork", bufs=1) as xwork,
            tc.tile_pool(name="qxp", bufs=1) as qxp,
            tc.tile_pool(name="wload", bufs=5) as wload,
            tc.tile_pool(name="wq", bufs=4) as wqp,
            tc.tile_pool(name="qwT", bufs=2) as qwTp,
            tc.tile_pool(name="bcast", bufs=1) as bcast,
            tc.tile_pool(name="outp", bufs=3) as outp,
            tc.tile_pool(name="consts", bufs=1) as consts,
            tc.tile_pool(name="psum", bufs=2, space=bass.MemorySpace.PSUM) as psum,
        ):
            qxT = consts.tile([128, n_kt, n_tok], BF16)
            sx_all = consts.tile([128, n_tt], F32)
            xsc = consts.tile([128, n_tt, 3], F32)   # m, r, r8 per token tile
            NQ = 4                                   # w streamed in quarter tiles
            KQ = K // NQ
            wsc = consts.tile([128, 16, NQ + 2], F32)  # q-maxes, mmax/rw, sw

            def w_prep(oc):
                """load + absmax + quant + transpose one output chunk -> qwT tile.

                Streamed at quarter-tile granularity so ~5 pieces pipeline
                through load->reduce->quant->transpose and handoff latency
                doesn't pace the chain."""
                qwT = qwTp.tile([128, n_kt, oc_size], BF16, tag="qwT")
                for j in range(ot_per_oc):
                    ot = oc * ot_per_oc + j
                    slot = ot % 16
                    wts = []
                    for q in range(NQ):
                        wt = wload.tile([128, KQ], F32, tag="wt")
                        nc.scalar.dma_start(
                            wt[:],
                            w_d.ap()[ot * 128:(ot + 1) * 128, q * KQ:(q + 1) * KQ],
                        )
                        nc.vector.tensor_reduce(
                            out=wsc[:, slot, q:q + 1], in_=wt[:],
                            axis=mybir.AxisListType.X,
                            op=mybir.AluOpType.max, apply_absolute_value=True,
                        )
                        wts.append(wt)
                    nc.vector.tensor_reduce(
                        out=wsc[:, slot, NQ:NQ + 1], in_=wsc[:, slot, 0:NQ],
                        axis=mybir.AxisListType.X, op=mybir.AluOpType.max,
                    )
                    sw = wsc[:, slot, NQ + 1:NQ + 2]
                    nc.vector.tensor_scalar(
                        out=sw, in0=wsc[:, slot, NQ:NQ + 1],
                        scalar1=float(np.float32(1.0) / np.float32(127.0)),
                        scalar2=1e-5,
                        op0=mybir.AluOpType.mult, op1=mybir.AluOpType.max,
                    )
                    nc.gpsimd.dma_start(swsc_d.ap()[ot * 128:(ot + 1) * 128], sw)
                    rw = wsc[:, slot, NQ:NQ + 1]  # overwrite mmax with 1/sw
                    nc.vector.reciprocal(rw, sw)
                    # weight is pre-quantized: w*rw lands within 5e-5 of an integer,
                    # so the bf16 output conversion rounds exactly onto the grid.
                    nktq = n_kt // NQ
                    for q in range(NQ):
                        qw = wqp.tile([128, KQ], BF16, tag="qw")
                        nc.scalar.activation(
                            out=qw[:], in_=wts[q][:],
                            func=mybir.ActivationFunctionType.Copy, bias=0.0, scale=rw,
                        )
                        nc.sync.dma_start_transpose(
                            qwT[:, q * nktq:(q + 1) * nktq, j * 128:(j + 1) * 128],
                            qw[:],
                        )
                # sw/bias broadcast tiles for this chunk
                swb = bcast.tile([128, oc_size], F32, tag="swb")
                src = swsc_d.ap()[oc * oc_size:(oc + 1) * oc_size]
                nc.gpsimd.dma_start(
                    out=swb[:],
                    in_=bass.AP(tensor=src.tensor, offset=src.offset,
                                ap=[[0, 128]] + list(src.ap)),
                )
                bb = bcast.tile([128, oc_size], F32, tag="bb")
                srcb = b_d.ap()[oc * oc_size:(oc + 1) * oc_size]
                nc.gpsimd.dma_start(
                    out=bb[:],
                    in_=bass.AP(tensor=srcb.tensor, offset=srcb.offset,
                                ap=[[0, 128]] + list(srcb.ap)),
                )
                return qwT, swb, bb

            def x_prep(tt):
                za = xload.tile([128, K], F32, tag="za")
                nc.scalar.dma_start(za[:], x_d.ap()[tt * 128:(tt + 1) * 128, :])
                zb = xwork.tile([128, K], F32, tag="zb")
                bufs = [za, zb]
                for s in range(6):
                    src, dst = bufs[s % 2], bufs[(s + 1) % 2]
                    blk = had_dim << s
                    sv = src[:].rearrange("p (a c b) -> p a c b", c=2, b=blk)
                    dv = dst[:].rearrange("p (a c b) -> p a c b", c=2, b=blk)
                    nc.vector.tensor_add(dv[:, :, 0, :], sv[:, :, 0, :], sv[:, :, 1, :])
                    nc.vector.tensor_sub(dv[:, :, 1, :], sv[:, :, 0, :], sv[:, :, 1, :])
                m = xsc[:, tt, 0:1]
                nc.vector.tensor_reduce(
                    out=m, in_=za[:], axis=mybir.AxisListType.X,
                    op=mybir.AluOpType.max, apply_absolute_value=True,
                )
                nc.vector.tensor_scalar(
                    out=sx_all[:, tt:tt + 1], in0=m,
                    scalar1=float(np.float32(1.0) / np.float32(1016.0)),
                    scalar2=1e-5,
                    op0=mybir.AluOpType.mult, op1=mybir.AluOpType.max,
                )
                rx = xsc[:, tt, 1:2]
                nc.vector.reciprocal(rx, sx_all[:, tt:tt + 1])
                rx8 = xsc[:, tt, 2:3]
                nc.vector.tensor_scalar_mul(rx8, rx, 0.125)
                qtmp = xwork.tile([128, K], F32, tag="zb")
                nc.scalar.activation(
                    out=qtmp[:], in_=za[:], func=mybir.ActivationFunctionType.Copy,
                    bias=MAGIC, scale=rx8,
                )
                qx = qxp.tile([128, K], BF16, tag="qx")
                nc.vector.tensor_scalar_add(qx[:], qtmp[:], -MAGIC)
                nc.sync.dma_start_transpose(
                    qxT[:, :, tt * 128:(tt + 1) * 128], qx[:]
                )

            def mm_chunk(oc, qwT, swb, bb):
                for t in range(n_tt):
                    ps = psum.tile([128, oc_size], F32, tag=f"ps{t % 4}")
                    for k in range(n_kt):
                        nc.tensor.matmul(
                            ps[:],
                            qxT[:, k, t * 128:(t + 1) * 128],
                            qwT[:, k, :],
                            start=(k == 0), stop=(k == n_kt - 1),
                        )
                    o_sb = outp.tile([128, oc_size], F32, tag="osb")
                    nc.vector.scalar_tensor_tensor(
                        out=o_sb[:], in0=ps[:], scalar=sx_all[:, t:t + 1], in1=swb[:],
                        op0=mybir.AluOpType.mult, op1=mybir.AluOpType.mult,
                    )
                    nc.gpsimd.tensor_add(o_sb[:], o_sb[:], bb[:])
                    nc.gpsimd.dma_start(
                        out_d.ap()[t * 128:(t + 1) * 128,
                                   oc * oc_size:(oc + 1) * oc_size],
                        o_sb[:],
                    )

            # emission order: interleave x tiles with the first weight chunks so
            # the x transposes hit the xbar ring first and qwT0 follows closely.
            x_prep(0)
            pend = [w_prep(0)]
            if n_tt > 1:
                x_prep(1)
            if n_oc > 1:
                pend.append(w_prep(1))
            for tt in range(2, n_tt):
                x_prep(tt)
            for oc in range(n_oc):
                qwT, swb, bb = pend.pop(0)
                mm_chunk(oc, qwT, swb, bb)
                if oc + 2 < n_oc:
                    pend.append(w_prep(oc + 2))

    nc.compile()
    return nc


_CACHED = None


def _get_full_kernel():
    global _CACHED
    if _CACHED is None:
        _CACHED = build_kernel(T_CORE, D_IN, D_OUT, 512)
    return _CACHED


def kernel(x, weight, bias, had_dim):
    assert int(had_dim) == 64
    assert x.shape == (B, S, D_IN) and weight.shape == (D_OUT, D_IN)
    nc = _get_full_kernel()
    xf = np.ascontiguousarray(np.asarray(x).reshape(N_TOK, D_IN), dtype=np.float32)
    w = np.ascontiguousarray(np.asarray(weight), dtype=np.float32)
    bi = np.ascontiguousarray(np.asarray(bias), dtype=np.float32)
    in_maps = [
        {"x": xf[i * T_CORE:(i + 1) * T_CORE], "w": w, "b": bi}
        for i in range(N_CORES)
    ]
    res = run_bass_kernel_spmd(nc, in_maps, core_ids=list(range(N_CORES)))
    out = np.concatenate([r["out"] for r in res.results], axis=0)
    return out.reshape(B, S, D_OUT)


if __name__ == "__main__":
    rng = np.random.default_rng(0)
    x = rng.standard_normal((B, S, D_IN), dtype=np.float32)
    w = rng.standard_normal((D_OUT, D_IN), dtype=np.float32)
    b = rng.standard_normal(D_OUT).astype(np.float32)
    o = kernel(x, w, b, np.int64(64))
    print(o.shape, o.dtype)


# revision 20
# speedup vs baseline: 1.4101x; 1.3247x over previous
"""Trainium2 Bass kernel for nn_ActQuantWrapper (hadamard + per-token act quant + linear).

Math (per reference):
  z = (H_64 kron I_had) x / 8               -- FHT over 64 groups along feature dim
  sx[t] = clip(absmax(z[t,:])/127, 1e-5)    -- per-token scale
  xq = round(z/sx)*sx                        -- act quant-dequant
  out = xq @ weight.T + bias                 -- weight already per-channel quantized

Device strategy (8 cores, data-parallel over tokens, weight replicated):
  - qx = round(z/sx) and qw = round(w/sw) are integers in [-127,127]: exactly
    representable in bf16, so the matmul runs at full bf16 PE rate and the
    result is scaled by sx[t]*sw[o] afterward (near-exact numerics).
  - The weight arrives already quantized, so bf16(w * (1/sw)) lands exactly on
    the integer grid without explicit rounding.
  - Activation rounding uses the fp32 magic-number trick (+1.5*2^23, -1.5*2^23).
  - bf16 tensors are transposed k-major via DMA xbar transpose (single ring --
    concurrent transposes on both HWDGE rings corrupt data).
  - The weight stream (load -> absmax -> quant -> transpose) is latency-bound
    (~6 cross-engine handoffs per tile), so after the x phase releases its SBUF
    the weight pipeline runs 5 full tiles deep to hide that latency.
"""

import numpy as np

import concourse.bass as bass
import concourse.tile as tile
from concourse import bacc, mybir
from concourse.bass_utils import run_bass_kernel_spmd

F32 = mybir.dt.float32
BF16 = mybir.dt.bfloat16
MAGIC = 12582912.0  # 1.5 * 2**23: adding then subtracting rounds f32 to int (RNE)

N_CORES = 8
B, S, D_IN, D_OUT = 2, 2048, 4096, 4096
N_TOK = B * S
T_CORE = N_TOK // N_CORES  # 512 tokens per core
N_GROUPS = 64              # hadamard dimension (fixed by reference)


def build_kernel(n_tok, K, O, oc_size, trace_sim=False):
    """Build + compile the per-core kernel."""
    assert n_tok % 128 == 0 and K % 512 == 0 and O % oc_size == 0
    assert oc_size % 128 == 0
    n_tt = n_tok // 128     # token tiles
    n_kt = K // 128         # contraction tiles
    n_oc = O // oc_size     # output chunks
    ot_per_oc = oc_size // 128
    had_dim = K // N_GROUPS
    NQ = 4                  # quant/transpose quarter granularity
    KQ = K // NQ
    nktq = n_kt // NQ

    nc = bacc.Bacc("TRN2", target_bir_lowering=False, debug=False)
    x_d = nc.dram_tensor("x", [n_tok, K], F32, kind="ExternalInput")
    w_d = nc.dram_tensor("w", [O, K], F32, kind="ExternalInput")
    b_d = nc.dram_tensor("b", [O], F32, kind="ExternalInput")
    out_d = nc.dram_tensor("out", [n_tok, O], F32, kind="ExternalOutput")
    swsc_d = nc.dram_tensor("swsc", [O], F32)  # scratch for sw broadcast

    with tile.TileContext(nc, trace_sim=trace_sim) as tc:
        with (
            tc.tile_pool(name="consts", bufs=1) as consts,
            tc.tile_pool(name="wq", bufs=4) as wqp,
            tc.tile_pool(name="qwT", bufs=2) as qwTp,
            tc.tile_pool(name="bcast", bufs=1) as bcast,
            tc.tile_pool(name="outp", bufs=2) as outp,
            tc.tile_pool(name="psum", bufs=2, space=bass.MemorySpace.PSUM) as psum,
        ):
            qxT = consts.tile([128, n_kt, n_tok], BF16)
            sx_all = consts.tile([128, n_tt], F32)
            xsc = consts.tile([128, n_tt, 3], F32)     # m, r, r8 per token tile
            wsc = consts.tile([128, 32, 3], F32)       # m, sw, rw per o-tile
            wpart = consts.tile([128, 2, NQ], F32)     # early-path partial maxes

            def w_scale_tail(ot):
                sw = wsc[:, ot % 32, 1:2]
                nc.vector.tensor_scalar(
                    out=sw, in0=wsc[:, ot % 32, 0:1],
                    scalar1=float(np.float32(1.0) / np.float32(127.0)),
                    scalar2=1e-5,
                    op0=mybir.AluOpType.mult, op1=mybir.AluOpType.max,
                )
                nc.gpsimd.dma_start(swsc_d.ap()[ot * 128:(ot + 1) * 128], sw)
                nc.vector.reciprocal(wsc[:, ot % 32, 2:3], sw)

            def w_quant_transpose_q(ot, j, src_ap, q, qwT):
                """quant + transpose one K-quarter of an o-tile.

                Weight is pre-quantized: w*rw lands within 5e-5 of an integer,
                so the bf16 output conversion rounds exactly onto the grid."""
                rw = wsc[:, ot % 32, 2:3]
                qw = wqp.tile([128, KQ], BF16, tag="qw")
                nc.scalar.activation(
                    out=qw[:], in_=src_ap,
                    func=mybir.ActivationFunctionType.Copy, bias=0.0, scale=rw,
                )
                nc.sync.dma_start_transpose(
                    qwT[:, q * nktq:(q + 1) * nktq, j * 128:(j + 1) * 128],
                    qw[:],
                )

            def w_bcasts(oc):
                swb = bcast.tile([128, oc_size], F32, tag="swb")
                src = swsc_d.ap()[oc * oc_size:(oc + 1) * oc_size]
                nc.gpsimd.dma_start(
                    out=swb[:],
                    in_=bass.AP(tensor=src.tensor, offset=src.offset,
                                ap=[[0, 128]] + list(src.ap)),
                )
                bb = bcast.tile([128, oc_size], F32, tag="bb")
                srcb = b_d.ap()[oc * oc_size:(oc + 1) * oc_size]
                nc.gpsimd.dma_start(
                    out=bb[:],
                    in_=bass.AP(tensor=srcb.tensor, offset=srcb.offset,
                                ap=[[0, 128]] + list(srcb.ap)),
                )
                return swb, bb

            def mm_chunk(oc, qwT, swb, bb):
                for t in range(n_tt):
                    ps = psum.tile([128, oc_size], F32, tag=f"ps{t % 4}")
                    for k in range(n_kt):
                        nc.tensor.matmul(
                            ps[:],
                            qxT[:, k, t * 128:(t + 1) * 128],
                            qwT[:, k, :],
                            start=(k == 0), stop=(k == n_kt - 1),
                        )
                    o_sb = outp.tile([128, oc_size], F32, tag="osb")
                    nc.vector.scalar_tensor_tensor(
                        out=o_sb[:], in0=ps[:], scalar=sx_all[:, t:t + 1],
                        in1=swb[:],
                        op0=mybir.AluOpType.mult, op1=mybir.AluOpType.mult,
                    )
                    nc.gpsimd.tensor_add(o_sb[:], o_sb[:], bb[:])
                    nc.gpsimd.dma_start(
                        out_d.ap()[t * 128:(t + 1) * 128,
                                   oc * oc_size:(oc + 1) * oc_size],
                        o_sb[:],
                    )

            # ---- startup phase: x prep + first two weight chunks (shallow) ----
            pend = []
            with (
                tc.tile_pool(name="xload", bufs=2) as xload,
                tc.tile_pool(name="xwork", bufs=1) as xwork,
                tc.tile_pool(name="qxp", bufs=1) as qxp,
                tc.tile_pool(name="wtE", bufs=5) as wtE,
            ):
                def x_prep(tt):
                    za = xload.tile([128, K], F32, tag="za")
                    nc.sync.dma_start(za[:], x_d.ap()[tt * 128:(tt + 1) * 128, :])
                    zb = xwork.tile([128, K], F32, tag="zb")
                    bufs = [za, zb]
                    for s in range(6):
                        src, dst = bufs[s % 2], bufs[(s + 1) % 2]
                        blk = had_dim << s
                        sv = src[:].rearrange("p (a c b) -> p a c b", c=2, b=blk)
                        dv = dst[:].rearrange("p (a c b) -> p a c b", c=2, b=blk)
                        nc.vector.tensor_add(
                            dv[:, :, 0, :], sv[:, :, 0, :], sv[:, :, 1, :])
                        nc.vector.tensor_sub(
                            dv[:, :, 1, :], sv[:, :, 0, :], sv[:, :, 1, :])
                    m = xsc[:, tt, 0:1]
                    nc.vector.tensor_reduce(
                        out=m, in_=za[:], axis=mybir.AxisListType.X,
                        op=mybir.AluOpType.max, apply_absolute_value=True,
                    )
                    nc.vector.tensor_scalar(
                        out=sx_all[:, tt:tt + 1], in0=m,
                        scalar1=float(np.float32(1.0) / np.float32(1016.0)),
                        scalar2=1e-5,
                        op0=mybir.AluOpType.mult, op1=mybir.AluOpType.max,
                    )
                    rx = xsc[:, tt, 1:2]
                    nc.vector.reciprocal(rx, sx_all[:, tt:tt + 1])
                    rx8 = xsc[:, tt, 2:3]
                    nc.vector.tensor_scalar_mul(rx8, rx, 0.125)
                    qtmp = xwork.tile([128, K], F32, tag="zb")
                    nc.scalar.activation(
                        out=qtmp[:], in_=za[:],
                        func=mybir.ActivationFunctionType.Copy,
                        bias=MAGIC, scale=rx8,
                    )
                    qx = qxp.tile([128, K], BF16, tag="qx")
                    nc.vector.tensor_scalar_add(qx[:], qtmp[:], -MAGIC)
                    nc.sync.dma_start_transpose(
                        qxT[:, :, tt * 128:(tt + 1) * 128], qx[:]
                    )

                def w_prep_early(oc):
                    """quarter-buffered weight chunk prep for the startup window"""
                    qwT = qwTp.tile([128, n_kt, oc_size], BF16, tag="qwT")
                    for j in range(ot_per_oc):
                        ot = oc * ot_per_oc + j
                        parts = wpart[:, ot % 2, :]
                        wts = []
                        for q in range(NQ):
                            wt = wtE.tile([128, KQ], F32, tag="wtE")
                            nc.sync.dma_start(
                                wt[:],
                                w_d.ap()[ot * 128:(ot + 1) * 128,
                                         q * KQ:(q + 1) * KQ],
                            )
                            nc.vector.tensor_reduce(
                                out=parts[:, q:q + 1], in_=wt[:],
                                axis=mybir.AxisListType.X,
                                op=mybir.AluOpType.max,
                                apply_absolute_value=True,
                            )
                            wts.append(wt)
                        nc.vector.tensor_reduce(
                            out=wsc[:, ot % 32, 0:1], in_=parts,
                            axis=mybir.AxisListType.X, op=mybir.AluOpType.max,
                        )
                        w_scale_tail(ot)
                        for q in range(NQ):
                            w_quant_transpose_q(ot, j, wts[q][:], q, qwT)
                    return (qwT, *w_bcasts(oc))

                x_prep(0)
                pend.append(w_prep_early(0))
                if n_tt > 1:
                    x_prep(1)
                if n_oc > 1:
                    pend.append(w_prep_early(1))
                for tt in range(2, n_tt):
                    x_prep(tt)

            # ---- steady phase: deep weight pipeline + matmuls ----
            with tc.tile_pool(name="wtB", bufs=5) as wtB:
                def w_prep(oc):
                    qwT = qwTp.tile([128, n_kt, oc_size], BF16, tag="qwT")
                    for j in range(ot_per_oc):
                        ot = oc * ot_per_oc + j
                        wt = wtB.tile([128, K], F32, tag="wtB")
                        nc.sync.dma_start(
                            wt[:], w_d.ap()[ot * 128:(ot + 1) * 128, :])
                        nc.vector.tensor_reduce(
                            out=wsc[:, ot % 32, 0:1], in_=wt[:],
                            axis=mybir.AxisListType.X,
                            op=mybir.AluOpType.max, apply_absolute_value=True,
                        )
                        w_scale_tail(ot)
                        for q in range(NQ):
                            w_quant_transpose_q(
                                ot, j, wt[:, q * KQ:(q + 1) * KQ], q, qwT)
                    return (qwT, *w_bcasts(oc))

                for oc in range(n_oc):
                    qwT, swb, bb = pend.pop(0)
                    mm_chunk(oc, qwT, swb, bb)
                    if oc + 2 < n_oc:
                        pend.append(w_prep(oc + 2))

    nc.compile()
    return nc


_CACHED = None


def _get_full_kernel():
    global _CACHED
    if _CACHED is None:
        _CACHED = build_kernel(T_CORE, D_IN, D_OUT, 512)
    return _CACHED


def kernel(x, weight, bias, had_dim):
    assert int(had_dim) == 64
    assert x.shape == (B, S, D_IN) and weight.shape == (D_OUT, D_IN)
    nc = _get_full_kernel()
    xf = np.ascontiguousarray(np.asarray(x).reshape(N_TOK, D_IN), dtype=np.float32)
    w = np.ascontiguousarray(np.asarray(weight), dtype=np.float32)
    bi = np.ascontiguousarray(np.asarray(bias), dtype=np.float32)
    in_maps = [
        {"x": xf[i * T_CORE:(i + 1) * T_CORE], "w": w, "b": bi}
        for i in range(N_CORES)
    ]
    res = run_bass_kernel_spmd(nc, in_maps, core_ids=list(range(N_CORES)))
    out = np.concatenate([r["out"] for r in res.results], axis=0)
    return out.reshape(B, S, D_OUT)


if __name__ == "__main__":
    rng = np.random.default_rng(0)
    x = rng.standard_normal((B, S, D_IN), dtype=np.float32)
    w = rng.standard_normal((D_OUT, D_IN), dtype=np.float32)
    b = rng.standard_normal(D_OUT).astype(np.float32)
    o = kernel(x, w, b, np.int64(64))
    print(o.shape, o.dtype)


# revision 21
# speedup vs baseline: 1.4970x; 1.0616x over previous
"""Trainium2 Bass kernel for nn_ActQuantWrapper (hadamard + per-token act quant + linear).

Math (per reference):
  z = (H_64 kron I_had) x / 8               -- FHT over 64 groups along feature dim
  sx[t] = clip(absmax(z[t,:])/127, 1e-5)    -- per-token scale
  xq = round(z/sx)*sx                        -- act quant-dequant
  out = xq @ weight.T + bias                 -- weight already per-channel quantized

Device strategy (8 cores, data-parallel over tokens, weight replicated):
  - qx = round(z/sx) and qw = round(w/sw) are integers in [-127,127]: exactly
    representable in bf16, so the matmul runs at full bf16 PE rate and the
    result is scaled by sx[t]*sw[o] afterward (near-exact numerics).
  - The weight arrives already quantized, so bf16(w * (1/sw)) lands exactly on
    the integer grid without explicit rounding.
  - Activation rounding uses the fp32 magic-number trick (+1.5*2^23, -1.5*2^23).
  - bf16 tensors are transposed k-major via DMA xbar transpose.
"""

import numpy as np

import concourse.bass as bass
import concourse.tile as tile
from concourse import bacc, mybir
from concourse.bass_utils import run_bass_kernel_spmd

F32 = mybir.dt.float32
BF16 = mybir.dt.bfloat16
MAGIC = 12582912.0  # 1.5 * 2**23: adding then subtracting rounds f32 to int (RNE)

N_CORES = 8
B, S, D_IN, D_OUT = 2, 2048, 4096, 4096
N_TOK = B * S
T_CORE = N_TOK // N_CORES  # 512 tokens per core
N_GROUPS = 64              # hadamard dimension (fixed by reference)


def build_kernel(n_tok, K, O, oc_size, trace_sim=False):
    """Build + compile the per-core kernel.

    n_tok: tokens per core (multiple of 128)
    K:     in features  (N_GROUPS * had_dim, multiple of 256)
    O:     out features (multiple of oc_size)
    oc_size: output-chunk width for the matmul (multiple of 128, <= 512)
    """
    assert n_tok % 128 == 0 and K % 256 == 0 and O % oc_size == 0
    assert oc_size % 128 == 0
    n_tt = n_tok // 128     # token tiles
    n_kt = K // 128         # contraction tiles
    n_oc = O // oc_size     # output chunks
    ot_per_oc = oc_size // 128
    had_dim = K // N_GROUPS
    KH = K // 2             # weight half-tile width

    nc = bacc.Bacc("TRN2", target_bir_lowering=False, debug=False)
    x_d = nc.dram_tensor("x", [n_tok, K], F32, kind="ExternalInput")
    w_d = nc.dram_tensor("w", [O, K], F32, kind="ExternalInput")
    b_d = nc.dram_tensor("b", [O], F32, kind="ExternalInput")
    out_d = nc.dram_tensor("out", [n_tok, O], F32, kind="ExternalOutput")
    swsc_d = nc.dram_tensor("swsc", [O], F32)  # internal scratch for sw broadcast

    with tile.TileContext(nc, trace_sim=trace_sim) as tc:
        with (
            tc.tile_pool(name="xload", bufs=2) as xload,
            tc.tile_pool(name="xwork", bufs=1) as xwork,
            tc.tile_pool(name="qxp", bufs=1) as qxp,
            tc.tile_pool(name="wload", bufs=3) as wload,
            tc.tile_pool(name="wq", bufs=2) as wqp,
            tc.tile_pool(name="qwT", bufs=2) as qwTp,
            tc.tile_pool(name="bcast", bufs=2) as bcast,
            tc.tile_pool(name="outp", bufs=3) as outp,
            tc.tile_pool(name="consts", bufs=1) as consts,
            tc.tile_pool(name="psum", bufs=2, space=bass.MemorySpace.PSUM) as psum,
        ):
            qxT = consts.tile([128, n_kt, n_tok], BF16)
            sx_all = consts.tile([128, n_tt], F32)
            # small per-row scalars as slices of shared tiles (slot = index % depth)
            xsc = consts.tile([128, n_tt, 3], F32)          # m, r, r8 per token tile
            wsc = consts.tile([128, 8, 4], F32)             # m0, m1, mmax/rw, sw

            # ---------------- x path: FHT -> quant -> transpose ----------------
            for tt in range(n_tt):
                za = xload.tile([128, K], F32, tag="za")
                nc.sync.dma_start(za[:], x_d.ap()[tt * 128:(tt + 1) * 128, :])
                zb = xwork.tile([128, K], F32, tag="zb")
                bufs = [za, zb]
                for s in range(6):
                    src, dst = bufs[s % 2], bufs[(s + 1) % 2]
                    blk = had_dim << s
                    sv = src[:].rearrange("p (a c b) -> p a c b", c=2, b=blk)
                    dv = dst[:].rearrange("p (a c b) -> p a c b", c=2, b=blk)
                    nc.vector.tensor_add(dv[:, :, 0, :], sv[:, :, 0, :], sv[:, :, 1, :])
                    nc.vector.tensor_sub(dv[:, :, 1, :], sv[:, :, 0, :], sv[:, :, 1, :])
                # 6 stages end back in za (unscaled by 1/8; folded into the scale)
                m = xsc[:, tt, 0:1]
                nc.vector.tensor_reduce(
                    out=m, in_=za[:], axis=mybir.AxisListType.X,
                    op=mybir.AluOpType.max, apply_absolute_value=True,
                )
                # sx = clip((m/8)/127, 1e-5) = clip(m/1016, 1e-5); m/8 is exact
                nc.vector.tensor_scalar(
                    out=sx_all[:, tt:tt + 1], in0=m,
                    scalar1=float(np.float32(1.0) / np.float32(1016.0)),
                    scalar2=1e-5,
                    op0=mybir.AluOpType.mult, op1=mybir.AluOpType.max,
                )
                rx = xsc[:, tt, 1:2]
                nc.vector.reciprocal(rx, sx_all[:, tt:tt + 1])
                rx8 = xsc[:, tt, 2:3]
                nc.vector.tensor_scalar_mul(rx8, rx, 0.125)
                qtmp = xwork.tile([128, K], F32, tag="zb")
                nc.scalar.activation(
                    out=qtmp[:], in_=za[:], func=mybir.ActivationFunctionType.Copy,
                    bias=MAGIC, scale=rx8,
                )
                qx = qxp.tile([128, K], BF16, tag="qx")
                nc.vector.tensor_scalar_add(qx[:], qtmp[:], -MAGIC)
                nc.scalar.dma_start_transpose(
                    qxT[:, :, tt * 128:(tt + 1) * 128], qx[:]
                )

            # ---------------- weight path + matmul, per output chunk ----------------
            for oc in range(n_oc):
                qwT = qwTp.tile([128, n_kt, oc_size], BF16, tag="qwT")
                for j in range(ot_per_oc):
                    ot = oc * ot_per_oc + j
                    slot = ot % 8
                    # stream the o-tile in two K-halves to cut SBUF pressure
                    wt0 = wload.tile([128, KH], F32, tag="wt")
                    nc.sync.dma_start(wt0[:], w_d.ap()[ot * 128:(ot + 1) * 128, 0:KH])
                    wt1 = wload.tile([128, KH], F32, tag="wt")
                    nc.sync.dma_start(wt1[:], w_d.ap()[ot * 128:(ot + 1) * 128, KH:K])
                    nc.vector.tensor_reduce(
                        out=wsc[:, slot, 0:1], in_=wt0[:], axis=mybir.AxisListType.X,
                        op=mybir.AluOpType.max, apply_absolute_value=True,
                    )
                    nc.vector.tensor_reduce(
                        out=wsc[:, slot, 1:2], in_=wt1[:], axis=mybir.AxisListType.X,
                        op=mybir.AluOpType.max, apply_absolute_value=True,
                    )
                    nc.vector.tensor_reduce(
                        out=wsc[:, slot, 2:3], in_=wsc[:, slot, 0:2],
                        axis=mybir.AxisListType.X, op=mybir.AluOpType.max,
                    )
                    sw = wsc[:, slot, 3:4]
                    nc.vector.tensor_scalar(
                        out=sw, in0=wsc[:, slot, 2:3],
                        scalar1=float(np.float32(1.0) / np.float32(127.0)),
                        scalar2=1e-5,
                        op0=mybir.AluOpType.mult, op1=mybir.AluOpType.max,
                    )
                    nc.gpsimd.dma_start(swsc_d.ap()[ot * 128:(ot + 1) * 128], sw)
                    rw = wsc[:, slot, 2:3]  # overwrite mmax with 1/sw
                    nc.vector.reciprocal(rw, sw)
                    # weight is pre-quantized: w*rw lands within 5e-5 of an integer,
                    # so the bf16 output conversion rounds exactly onto the grid.
                    qw = wqp.tile([128, K], BF16, tag="qw")
                    nc.scalar.activation(
                        out=qw[:, 0:KH], in_=wt0[:],
                        func=mybir.ActivationFunctionType.Copy, bias=0.0, scale=rw,
                    )
                    nc.scalar.activation(
                        out=qw[:, KH:K], in_=wt1[:],
                        func=mybir.ActivationFunctionType.Copy, bias=0.0, scale=rw,
                    )
                    nc.scalar.dma_start_transpose(
                        qwT[:, 0:n_kt // 2, j * 128:(j + 1) * 128], qw[:, 0:KH]
                    )
                    nc.scalar.dma_start_transpose(
                        qwT[:, n_kt // 2:n_kt, j * 128:(j + 1) * 128], qw[:, KH:K]
                    )

                # sw/bias broadcast tiles for this chunk ([128, oc_size])
                swb = bcast.tile([128, oc_size], F32, tag="swb")
                src = swsc_d.ap()[oc * oc_size:(oc + 1) * oc_size]
                nc.gpsimd.dma_start(
                    out=swb[:],
                    in_=bass.AP(tensor=src.tensor, offset=src.offset,
                                ap=[[0, 128]] + list(src.ap)),
                )
                bb = bcast.tile([128, oc_size], F32, tag="bb")
                srcb = b_d.ap()[oc * oc_size:(oc + 1) * oc_size]
                nc.gpsimd.dma_start(
                    out=bb[:],
                    in_=bass.AP(tensor=srcb.tensor, offset=srcb.offset,
                                ap=[[0, 128]] + list(srcb.ap)),
                )

                for t in range(n_tt):
                    ps = psum.tile([128, oc_size], F32, tag=f"ps{t % 4}")
                    for k in range(n_kt):
                        nc.tensor.matmul(
                            ps[:],
                            qxT[:, k, t * 128:(t + 1) * 128],
                            qwT[:, k, :],
                            start=(k == 0), stop=(k == n_kt - 1),
                        )
                    o_sb = outp.tile([128, oc_size], F32, tag="osb")
                    # out = (psum * sx[t]) * swb + bias
                    nc.vector.scalar_tensor_tensor(
                        out=o_sb[:], in0=ps[:], scalar=sx_all[:, t:t + 1], in1=swb[:],
                        op0=mybir.AluOpType.mult, op1=mybir.AluOpType.mult,
                    )
                    nc.gpsimd.tensor_add(o_sb[:], o_sb[:], bb[:])
                    nc.gpsimd.dma_start(
                        out_d.ap()[t * 128:(t + 1) * 128,
                                   oc * oc_size:(oc + 1) * oc_size],
                        o_sb[:],
                    )

    nc.compile()
    return nc


_CACHED = None


def _get_full_kernel():
    global _CACHED
    if _CACHED is None:
        _CACHED = build_kernel(T_CORE, D_IN, D_OUT, 512)
    return _CACHED


def kernel(x, weight, bias, had_dim):
    assert int(had_dim) == 64
    assert x.shape == (B, S, D_IN) and weight.shape == (D_OUT, D_IN)
    nc = _get_full_kernel()
    xf = np.ascontiguousarray(np.asarray(x).reshape(N_TOK, D_IN), dtype=np.float32)
    w = np.ascontiguousarray(np.asarray(weight), dtype=np.float32)
    bi = np.ascontiguousarray(np.asarray(bias), dtype=np.float32)
    in_maps = [
        {"x": xf[i * T_CORE:(i + 1) * T_CORE], "w": w, "b": bi}
        for i in range(N_CORES)
    ]
    res = run_bass_kernel_spmd(nc, in_maps, core_ids=list(range(N_CORES)))
    out = np.concatenate([r["out"] for r in res.results], axis=0)
    return out.reshape(B, S, D_OUT)


if __name__ == "__main__":
    rng = np.random.default_rng(0)
    x = rng.standard_normal((B, S, D_IN), dtype=np.float32)
    w = rng.standard_normal((D_OUT, D_IN), dtype=np.float32)
    b = rng.standard_normal(D_OUT).astype(np.float32)
    o = kernel(x, w, b, np.int64(64))
    print(o.shape, o.dtype)
